# revision 28
# baseline (speedup 1.0000x reference)
"""Trainium2 Bass/Tile kernel for the HairBundle SDE drift+diffusion.

Contract: kernel(t, x) takes the FULL inputs (t: [1] f32, x: [8_000_000, 5]
f32) and returns the full (drift, diffusion) pair, matching reference().

Strategy
--------
Trivially data-parallel over the sample-path axis: 8 NeuronCores, each core
takes 1M rows padded to 128*7813.  The host hands each core PLANAR data
[128 partitions, 5 components, 7813 rows] (one numpy transpose each way) so
that every DMA is dense AND every on-chip access pattern is unit-stride --
strided (interleaved) operands run at half rate on both VectorE and
ScalarE, so de-interleaving on the host removes the whole tax.  Per tile
the drift is 9 contiguous VectorE streams + 6 ScalarE streams; loads issue
from the sync queue, stores from the gpsimd queue so neither blocks.
The diffusion output is a constant broadcast, produced host-side for free.

Math (constants folded from the reference):
    d  = h - a;  po = sigmoid(4 d)
    dh = 0.375*(2*(a - 1.8 h) + po) + force          (ACT bias = force)
    da = 0.0375*(2h - 3.2a - po + 0.84 m) - 0.035
    dv + k = (v - 1)*(-c*po - k)   for (v,c,k) in
         (m,1.2,0.8), (g,0.7,0.5), (t,0.3,0.4)   [host subtracts k]
    force = 0.5*sin(2*pi*t)
"""

import numpy as np

_B = 8_000_000
_NCORES = 8
_RPC = _B // _NCORES            # rows per core = 1_000_000
_P = 128
_Q = -(-_RPC // _P)             # 7813 rows per partition (padded by 64 rows)
_F = 512                        # rows-per-partition per SBUF tile
_DSIG = np.array([0.05, 0.02, 0.0, 0.0, 0.0], dtype=np.float32)

_CACHE = {}


def _build_nc(q, f):
    """Per-core Bass program: x [128, 5, q] planar -> drift [128, 5, q]."""
    import concourse.bacc as bacc
    import concourse.mybir as mybir
    import concourse.tile as tile

    f32 = mybir.dt.float32
    Act = mybir.ActivationFunctionType
    Op = mybir.AluOpType

    nc = bacc.Bacc("TRN2", debug=False)
    x_d = nc.dram_tensor("x", [_P, 5, q], f32, kind="ExternalInput").ap()
    c_d = nc.dram_tensor("consts", [_P, 5], f32, kind="ExternalInput").ap()
    o_d = nc.dram_tensor("drift", [_P, 5, q], f32, kind="ExternalOutput").ap()

    ntiles = -(-q // f)

    with tile.TileContext(nc) as tc:
        with (
            tc.tile_pool(name="io", bufs=3) as io_pool,
            tc.tile_pool(name="tmp", bufs=2) as tmp_pool,
            tc.tile_pool(name="cst", bufs=1) as cst_pool,
        ):
            consts = cst_pool.tile([_P, 5], f32, name="consts_sb")
            nc.sync.dma_start(consts[:, :], c_d[:, :])
            force_b = consts[:, 0:1]
            cprime_b = consts[:, 1:2]
            km_b = consts[:, 2:3]   # -0.8
            kg_b = consts[:, 3:4]   # -0.5
            kt_b = consts[:, 4:5]   # -0.4

            for ti in range(ntiles):
                f0 = ti * f
                fw = min(f, q - f0)

                X = io_pool.tile([_P, 5, f], f32, tag="X", name="X", bufs=4)
                nc.sync.dma_start(X[:, :, :fw], x_d[:, :, f0 : f0 + fw])
                D = io_pool.tile([_P, 5, f], f32, tag="D", name="D", bufs=3)

                h = X[:, 0, :fw]
                a = X[:, 1, :fw]
                m = X[:, 2, :fw]
                g = X[:, 3, :fw]
                t_ = X[:, 4, :fw]
                dh = D[:, 0, :fw]
                da = D[:, 1, :fw]
                dm = D[:, 2, :fw]
                dg = D[:, 3, :fw]
                dt = D[:, 4, :fw]

                def T(nm, bufs=2):
                    # bufs=1 for temps consumed on the same engine that wrote
                    # them (in-order engines make the WAR free)
                    return tmp_pool.tile([_P, f], f32, tag=nm, name=nm, bufs=bufs)[
                        :, :fw
                    ]

                d = T("d")
                po = T("po")
                u1 = T("u1")
                u2 = T("u2")
                zA = T("zA", 1)
                z2 = T("z2", 1)
                zp = T("zp")
                qm = T("qm")
                qg = T("qg")
                qt = T("qt")

                # d = h - a ; po = sigmoid(4 d)
                nc.vector.tensor_tensor(d, h, a, Op.subtract)
                nc.scalar.activation(po, d, Act.Sigmoid, scale=4.0)

                # dh = 0.375*(2*(a - 1.8 h) + po) + force
                nc.vector.scalar_tensor_tensor(u1, h, -1.8, a, Op.mult, Op.add)
                nc.vector.scalar_tensor_tensor(u2, u1, 2.0, po, Op.mult, Op.add)
                nc.scalar.activation(dh, u2, Act.Identity, bias=force_b, scale=0.375)

                # da = 0.0375*(2h - 3.2a - po + 0.84 m) - 0.035
                # 2h - 3.2a = 4.7 d + 1.5 u1
                nc.vector.scalar_tensor_tensor(zA, d, 4.7 / 1.5, u1, Op.mult, Op.add)
                nc.vector.scalar_tensor_tensor(z2, zA, 1.5, po, Op.mult, Op.subtract)
                nc.vector.scalar_tensor_tensor(zp, m, 0.84, z2, Op.mult, Op.add)
                nc.scalar.activation(da, zp, Act.Identity, bias=cprime_b, scale=0.0375)

                # dv + k = (v-1)*(-c po - k); host subtracts k after gather
                nc.scalar.activation(qm, po, Act.Identity, bias=km_b, scale=-1.2)
                nc.scalar.activation(qg, po, Act.Identity, bias=kg_b, scale=-0.7)
                nc.scalar.activation(qt, po, Act.Identity, bias=kt_b, scale=-0.3)
                nc.vector.scalar_tensor_tensor(dm, m, 1.0, qm, Op.subtract, Op.mult)
                nc.vector.scalar_tensor_tensor(dg, g, 1.0, qg, Op.subtract, Op.mult)
                nc.vector.scalar_tensor_tensor(dt, t_, 1.0, qt, Op.subtract, Op.mult)

                # out-DMA on the (otherwise idle) gpsimd SWDGE queue so its
                # wait-on-compute doesn't block the sync queue's in-DMAs
                nc.gpsimd.dma_start(o_d[:, :, f0 : f0 + fw], D[:, :, :fw])

    nc.compile()
    return nc


def _get_nc():
    key = (_Q, _F)
    if key not in _CACHE:
        _CACHE[key] = _build_nc(_Q, _F)
    return _CACHE[key]


def _run_device(x, force, trace=False, tmpdir=None):
    """Shard x [8M,5] over 8 cores (planar per-core layout), gather drift."""
    from concourse.bass_utils import run_bass_kernel_spmd

    nc = _get_nc()

    consts_np = np.empty((_P, 5), dtype=np.float32)
    consts_np[:, 0] = force
    consts_np[:, 1] = -0.035
    consts_np[:, 2] = -0.8
    consts_np[:, 3] = -0.5
    consts_np[:, 4] = -0.4

    in_maps = []
    for i in range(_NCORES):
        shard = np.zeros((_P, _Q, 5), dtype=np.float32)
        shard.reshape(_P * _Q, 5)[:_RPC] = x[i * _RPC : (i + 1) * _RPC]
        planar = np.ascontiguousarray(shard.transpose(0, 2, 1))  # [P, 5, Q]
        in_maps.append({"x": planar, "consts": consts_np})

    res = run_bass_kernel_spmd(
        nc, in_maps, list(range(_NCORES)), trace=trace, tmpdir=tmpdir
    )

    drift = np.empty((_B, 5), dtype=np.float32)
    for i in range(_NCORES):
        out = res.results[i]["drift"]  # [P, 5, Q] planar
        rows = out.transpose(0, 2, 1).reshape(_P * _Q, 5)
        drift[i * _RPC : (i + 1) * _RPC] = rows[:_RPC]
    # device leaves channels 2..4 k-shifted by (0.8, 0.5, 0.4)
    drift[:, 2] -= np.float32(0.8)
    drift[:, 3] -= np.float32(0.5)
    drift[:, 4] -= np.float32(0.4)
    return drift, res


def kernel(t, x):
    t = np.asarray(t, dtype=np.float32)
    x = np.asarray(x, dtype=np.float32)
    force = np.float32(0.5 * np.sin(6.283185307179586 * float(t[0]) + 0.0))
    drift, _ = _run_device(x, force, trace=False)
    diffusion = np.broadcast_to(_DSIG, x.shape)
    return drift, diffusion


# revision 30
# speedup vs baseline: 1.2382x; 1.2382x over previous
"""Trainium2 Bass/Tile kernel for the HairBundle SDE drift+diffusion.

Contract: kernel(t, x) takes the FULL inputs (t: [1] f32, x: [8_000_000, 5]
f32) and returns the full (drift, diffusion) pair, matching reference().

Strategy
--------
Trivially data-parallel over the sample-path axis: 8 NeuronCores, each core
takes 1M rows padded to 128*7813.  The host hands each core PLANAR data
[128 partitions, 5 components, 7813 rows] (one numpy transpose each way) so
that every DMA is dense AND every on-chip access pattern is unit-stride --
strided (interleaved) operands run at half rate on both VectorE and
ScalarE, so de-interleaving on the host removes the whole tax.  Per tile
the drift is 9 contiguous VectorE streams + 6 ScalarE streams; loads issue
from the sync queue, stores from the gpsimd queue so neither blocks.
The diffusion output is a constant broadcast, produced host-side for free.

Math (constants folded from the reference):
    d  = h - a;  po = sigmoid(4 d)
    dh = 0.375*(2*(a - 1.8 h) + po) + force          (ACT bias = force)
    da = 0.0375*(2h - 3.2a - po + 0.84 m) - 0.035
    dv + k = (v - 1)*(-c*po - k)   for (v,c,k) in
         (m,1.2,0.8), (g,0.7,0.5), (t,0.3,0.4)   [host subtracts k]
    force = 0.5*sin(2*pi*t)
"""

import numpy as np

_B = 8_000_000
_NCORES = 8
_RPC = _B // _NCORES            # rows per core = 1_000_000
_P = 128
_Q = -(-_RPC // _P)             # 7813 rows per partition (padded by 64 rows)
_F = 1024                       # rows-per-partition per SBUF tile
_DSIG = np.array([0.05, 0.02, 0.0, 0.0, 0.0], dtype=np.float32)

_CACHE = {}


def _build_nc(q, f):
    """Per-core Bass program: x [128, 5, q] planar -> drift [128, 5, q]."""
    import concourse.bacc as bacc
    import concourse.mybir as mybir
    import concourse.tile as tile

    f32 = mybir.dt.float32
    Act = mybir.ActivationFunctionType
    Op = mybir.AluOpType

    nc = bacc.Bacc("TRN2", debug=False)
    x_d = nc.dram_tensor("x", [_P, 5, q], f32, kind="ExternalInput").ap()
    c_d = nc.dram_tensor("consts", [_P, 5], f32, kind="ExternalInput").ap()
    o_d = nc.dram_tensor("drift", [_P, 5, q], f32, kind="ExternalOutput").ap()

    # tapered schedule: small first tile for a fast pipeline ramp, small
    # tail tiles so the final compute+store drain is short
    widths = []
    rem = q
    if rem > 2 * f and f >= 1024:
        widths.append(512)
        rem -= 512
    while rem > max(f, 1669 if f >= 1024 else 0):
        widths.append(f)
        rem -= f
    if f >= 1024:
        for w in (768, 640, 512):
            if rem > w:
                widths.append(w)
                rem -= w
    while rem > f:
        widths.append(f)
        rem -= f
    if rem:
        widths.append(rem)
    assert sum(widths) == q and max(widths) <= max(f, 512)

    with tile.TileContext(nc) as tc:
        with (
            tc.tile_pool(name="io", bufs=3) as io_pool,
            tc.tile_pool(name="tmp", bufs=2) as tmp_pool,
            tc.tile_pool(name="cst", bufs=1) as cst_pool,
        ):
            consts = cst_pool.tile([_P, 5], f32, name="consts_sb")
            nc.sync.dma_start(consts[:, :], c_d[:, :])
            force_b = consts[:, 0:1]
            cprime_b = consts[:, 1:2]
            km_b = consts[:, 2:3]   # -0.8
            kg_b = consts[:, 3:4]   # -0.5
            kt_b = consts[:, 4:5]   # -0.4

            f0 = 0
            for ti, fw in enumerate(widths):

                X = io_pool.tile([_P, 5, f], f32, tag="X", name="X", bufs=3)
                nc.sync.dma_start(X[:, :, :fw], x_d[:, :, f0 : f0 + fw])
                D = io_pool.tile([_P, 5, f], f32, tag="D", name="D", bufs=3)

                h = X[:, 0, :fw]
                a = X[:, 1, :fw]
                m = X[:, 2, :fw]
                g = X[:, 3, :fw]
                t_ = X[:, 4, :fw]
                dh = D[:, 0, :fw]
                da = D[:, 1, :fw]
                dm = D[:, 2, :fw]
                dg = D[:, 3, :fw]
                dt = D[:, 4, :fw]

                def T(nm, bufs=2):
                    # bufs=1 for temps consumed on the same engine that wrote
                    # them (in-order engines make the WAR free)
                    return tmp_pool.tile([_P, f], f32, tag=nm, name=nm, bufs=bufs)[
                        :, :fw
                    ]

                d = T("d")
                po = T("po")
                u1 = T("u1")
                u2 = T("u2")
                zA = T("zA", 1)
                z2 = T("z2", 1)
                zp = T("zp")
                qm = T("qm")
                qg = T("qg")
                qt = T("qt")

                # d = h - a ; po = sigmoid(4 d)
                nc.vector.tensor_tensor(d, h, a, Op.subtract)
                nc.scalar.activation(po, d, Act.Sigmoid, scale=4.0)

                # dh = 0.375*(2*(a - 1.8 h) + po) + force
                nc.vector.scalar_tensor_tensor(u1, h, -1.8, a, Op.mult, Op.add)
                nc.vector.scalar_tensor_tensor(u2, u1, 2.0, po, Op.mult, Op.add)
                nc.scalar.activation(dh, u2, Act.Identity, bias=force_b, scale=0.375)

                # da = 0.0375*(2h - 3.2a - po + 0.84 m) - 0.035
                # 2h - 3.2a = 4.7 d + 1.5 u1
                nc.vector.scalar_tensor_tensor(zA, d, 4.7 / 1.5, u1, Op.mult, Op.add)
                nc.vector.scalar_tensor_tensor(z2, zA, 1.5, po, Op.mult, Op.subtract)
                nc.vector.scalar_tensor_tensor(zp, m, 0.84, z2, Op.mult, Op.add)
                nc.scalar.activation(da, zp, Act.Identity, bias=cprime_b, scale=0.0375)

                # dv + k = (v-1)*(-c po - k); host subtracts k after gather
                nc.scalar.activation(qm, po, Act.Identity, bias=km_b, scale=-1.2)
                nc.scalar.activation(qg, po, Act.Identity, bias=kg_b, scale=-0.7)
                nc.scalar.activation(qt, po, Act.Identity, bias=kt_b, scale=-0.3)
                nc.vector.scalar_tensor_tensor(dm, m, 1.0, qm, Op.subtract, Op.mult)
                nc.vector.scalar_tensor_tensor(dg, g, 1.0, qg, Op.subtract, Op.mult)
                nc.vector.scalar_tensor_tensor(dt, t_, 1.0, qt, Op.subtract, Op.mult)

                # out-DMA from the ACT HWDGE ring: it issues right after
                # this tile's final D-writes without blocking the sync
                # queue's in-DMAs, and costs no gpsimd descriptor-gen time
                nc.scalar.dma_start(o_d[:, :, f0 : f0 + fw], D[:, :, :fw])
                f0 += fw

    nc.compile()
    return nc


def _get_nc():
    key = (_Q, _F)
    if key not in _CACHE:
        _CACHE[key] = _build_nc(_Q, _F)
    return _CACHE[key]


def _run_device(x, force, trace=False, tmpdir=None):
    """Shard x [8M,5] over 8 cores (planar per-core layout), gather drift."""
    from concourse.bass_utils import run_bass_kernel_spmd

    nc = _get_nc()

    consts_np = np.empty((_P, 5), dtype=np.float32)
    consts_np[:, 0] = force
    consts_np[:, 1] = -0.035
    consts_np[:, 2] = -0.8
    consts_np[:, 3] = -0.5
    consts_np[:, 4] = -0.4

    in_maps = []
    for i in range(_NCORES):
        shard = np.zeros((_P, _Q, 5), dtype=np.float32)
        shard.reshape(_P * _Q, 5)[:_RPC] = x[i * _RPC : (i + 1) * _RPC]
        planar = np.ascontiguousarray(shard.transpose(0, 2, 1))  # [P, 5, Q]
        in_maps.append({"x": planar, "consts": consts_np})

    res = run_bass_kernel_spmd(
        nc, in_maps, list(range(_NCORES)), trace=trace, tmpdir=tmpdir
    )

    drift = np.empty((_B, 5), dtype=np.float32)
    for i in range(_NCORES):
        out = res.results[i]["drift"]  # [P, 5, Q] planar
        rows = out.transpose(0, 2, 1).reshape(_P * _Q, 5)
        drift[i * _RPC : (i + 1) * _RPC] = rows[:_RPC]
    # device leaves channels 2..4 k-shifted by (0.8, 0.5, 0.4)
    drift[:, 2] -= np.float32(0.8)
    drift[:, 3] -= np.float32(0.5)
    drift[:, 4] -= np.float32(0.4)
    return drift, res


def kernel(t, x):
    t = np.asarray(t, dtype=np.float32)
    x = np.asarray(x, dtype=np.float32)
    force = np.float32(0.5 * np.sin(6.283185307179586 * float(t[0]) + 0.0))
    drift, _ = _run_device(x, force, trace=False)
    diffusion = np.broadcast_to(_DSIG, x.shape)
    return drift, diffusion


# revision 31
# speedup vs baseline: 1.3362x; 1.0792x over previous
"""Trainium2 Bass/Tile kernel for the HairBundle SDE drift+diffusion.

Contract: kernel(t, x) takes the FULL inputs (t: [1] f32, x: [8_000_000, 5]
f32) and returns the full (drift, diffusion) pair, matching reference().

Strategy
--------
Trivially data-parallel over the sample-path axis: 8 NeuronCores, each core
takes 1M rows padded to 128*7813.  The host hands each core PLANAR data
[128 partitions, 5 components, 7813 rows] (one numpy transpose each way) so
that every DMA is dense AND every on-chip access pattern is unit-stride --
strided (interleaved) operands run at half rate on both VectorE and
ScalarE, so de-interleaving on the host removes the whole tax.  Per tile
the drift is 9 contiguous VectorE streams + 6 ScalarE streams; loads issue
from the sync queue, stores from the gpsimd queue so neither blocks.
The diffusion output is a constant broadcast, produced host-side for free.

Math (constants folded from the reference):
    d  = h - a;  po = sigmoid(4 d)
    dh = 0.375*(2*(a - 1.8 h) + po) + force          (ACT bias = force)
    da = 0.0375*(2h - 3.2a - po + 0.84 m) - 0.035
    dv + k = (v - 1)*(-c*po - k)   for (v,c,k) in
         (m,1.2,0.8), (g,0.7,0.5), (t,0.3,0.4)   [host subtracts k]
    force = 0.5*sin(2*pi*t)
"""

import numpy as np

_B = 8_000_000
_NCORES = 8
_RPC = _B // _NCORES            # rows per core = 1_000_000
_P = 128
_Q = -(-_RPC // _P)             # 7813 rows per partition (padded by 64 rows)
_F = 1024                       # rows-per-partition per SBUF tile
_DSIG = np.array([0.05, 0.02, 0.0, 0.0, 0.0], dtype=np.float32)

_CACHE = {}


def _build_nc(q, f):
    """Per-core Bass program: x [128, 5, q] planar -> drift [128, 5, q]."""
    import concourse.bacc as bacc
    import concourse.mybir as mybir
    import concourse.tile as tile

    f32 = mybir.dt.float32
    Act = mybir.ActivationFunctionType
    Op = mybir.AluOpType

    nc = bacc.Bacc("TRN2", debug=False)
    x_d = nc.dram_tensor("x", [_P, 5, q], f32, kind="ExternalInput").ap()
    c_d = nc.dram_tensor("consts", [_P, 5], f32, kind="ExternalInput").ap()
    o_d = nc.dram_tensor("drift", [_P, 5, q], f32, kind="ExternalOutput").ap()

    # tapered schedule: small first tile for a fast pipeline ramp, small
    # tail tiles so the final compute+store drain is short
    widths = []
    rem = q
    if rem > 2 * f and f >= 1024:
        widths.append(512)
        rem -= 512
    while rem > max(f, 1669 if f >= 1024 else 0):
        widths.append(f)
        rem -= f
    if f >= 1024:
        for w in (768, 640, 512):
            if rem > w:
                widths.append(w)
                rem -= w
    while rem > f:
        widths.append(f)
        rem -= f
    if rem:
        widths.append(rem)
    assert sum(widths) == q and max(widths) <= max(f, 512)

    with tile.TileContext(nc) as tc:
        with (
            tc.tile_pool(name="io", bufs=3) as io_pool,
            tc.tile_pool(name="tmp", bufs=2) as tmp_pool,
            tc.tile_pool(name="cst", bufs=1) as cst_pool,
        ):
            consts = cst_pool.tile([_P, 5], f32, name="consts_sb")
            nc.sync.dma_start(consts[:, :], c_d[:, :])
            force_b = consts[:, 0:1]
            cprime_b = consts[:, 1:2]
            km_b = consts[:, 2:3]   # -0.8
            kg_b = consts[:, 3:4]   # -0.5
            kt_b = consts[:, 4:5]   # -0.4

            f0 = 0
            for ti, fw in enumerate(widths):

                X = io_pool.tile([_P, 5, f], f32, tag="X", name="X", bufs=3)
                nc.sync.dma_start(X[:, :, :fw], x_d[:, :, f0 : f0 + fw])
                D = io_pool.tile([_P, 5, f], f32, tag="D", name="D", bufs=3)

                h = X[:, 0, :fw]
                a = X[:, 1, :fw]
                m = X[:, 2, :fw]
                g = X[:, 3, :fw]
                t_ = X[:, 4, :fw]
                dh = D[:, 0, :fw]
                da = D[:, 1, :fw]
                dm = D[:, 2, :fw]
                dg = D[:, 3, :fw]
                dt = D[:, 4, :fw]

                def T(nm, bufs=2):
                    # bufs=1 for temps consumed on the same engine that wrote
                    # them (in-order engines make the WAR free)
                    return tmp_pool.tile([_P, f], f32, tag=nm, name=nm, bufs=bufs)[
                        :, :fw
                    ]

                d = T("d")
                po = T("po")
                u1 = T("u1")
                u2 = T("u2")
                zA = T("zA", 1)
                z2 = T("z2", 1)
                zp = T("zp")
                qm = T("qm")
                qg = T("qg")
                qt = T("qt")

                # d = h - a ; po = sigmoid(4 d)
                nc.vector.tensor_tensor(d, h, a, Op.subtract)
                nc.scalar.activation(po, d, Act.Sigmoid, scale=4.0)

                # dh = 0.375*(2*(a - 1.8 h) + po) + force
                nc.vector.scalar_tensor_tensor(u1, h, -1.8, a, Op.mult, Op.add)
                nc.vector.scalar_tensor_tensor(u2, u1, 2.0, po, Op.mult, Op.add)
                nc.scalar.activation(dh, u2, Act.Identity, bias=force_b, scale=0.375)

                # da = 0.0375*(2h - 3.2a - po + 0.84 m) - 0.035
                # 2h - 3.2a = 4.7 d + 1.5 u1
                nc.vector.scalar_tensor_tensor(zA, d, 4.7 / 1.5, u1, Op.mult, Op.add)
                nc.vector.scalar_tensor_tensor(z2, zA, 1.5, po, Op.mult, Op.subtract)
                nc.vector.scalar_tensor_tensor(zp, m, 0.84, z2, Op.mult, Op.add)
                nc.scalar.activation(da, zp, Act.Identity, bias=cprime_b, scale=0.0375)

                # dv + k = (v-1)*(-c po - k); host subtracts k after gather
                nc.scalar.activation(qm, po, Act.Identity, bias=km_b, scale=-1.2)
                nc.scalar.activation(qg, po, Act.Identity, bias=kg_b, scale=-0.7)
                nc.scalar.activation(qt, po, Act.Identity, bias=kt_b, scale=-0.3)
                nc.vector.scalar_tensor_tensor(dm, m, 1.0, qm, Op.subtract, Op.mult)
                nc.vector.scalar_tensor_tensor(dg, g, 1.0, qg, Op.subtract, Op.mult)
                nc.vector.scalar_tensor_tensor(dt, t_, 1.0, qt, Op.subtract, Op.mult)

                # out-DMA on the (otherwise idle) gpsimd SWDGE queue so its
                # wait-on-compute doesn't block the sync queue's in-DMAs
                nc.gpsimd.dma_start(o_d[:, :, f0 : f0 + fw], D[:, :, :fw])
                f0 += fw

    nc.compile()
    return nc


def _get_nc():
    key = (_Q, _F)
    if key not in _CACHE:
        _CACHE[key] = _build_nc(_Q, _F)
    return _CACHE[key]


def _run_device(x, force, trace=False, tmpdir=None):
    """Shard x [8M,5] over 8 cores (planar per-core layout), gather drift."""
    from concourse.bass_utils import run_bass_kernel_spmd

    nc = _get_nc()

    consts_np = np.empty((_P, 5), dtype=np.float32)
    consts_np[:, 0] = force
    consts_np[:, 1] = -0.035
    consts_np[:, 2] = -0.8
    consts_np[:, 3] = -0.5
    consts_np[:, 4] = -0.4

    in_maps = []
    for i in range(_NCORES):
        shard = np.zeros((_P, _Q, 5), dtype=np.float32)
        shard.reshape(_P * _Q, 5)[:_RPC] = x[i * _RPC : (i + 1) * _RPC]
        planar = np.ascontiguousarray(shard.transpose(0, 2, 1))  # [P, 5, Q]
        in_maps.append({"x": planar, "consts": consts_np})

    res = run_bass_kernel_spmd(
        nc, in_maps, list(range(_NCORES)), trace=trace, tmpdir=tmpdir
    )

    drift = np.empty((_B, 5), dtype=np.float32)
    for i in range(_NCORES):
        out = res.results[i]["drift"]  # [P, 5, Q] planar
        rows = out.transpose(0, 2, 1).reshape(_P * _Q, 5)
        drift[i * _RPC : (i + 1) * _RPC] = rows[:_RPC]
    # device leaves channels 2..4 k-shifted by (0.8, 0.5, 0.4)
    drift[:, 2] -= np.float32(0.8)
    drift[:, 3] -= np.float32(0.5)
    drift[:, 4] -= np.float32(0.4)
    return drift, res


def kernel(t, x):
    t = np.asarray(t, dtype=np.float32)
    x = np.asarray(x, dtype=np.float32)
    force = np.float32(0.5 * np.sin(6.283185307179586 * float(t[0]) + 0.0))
    drift, _ = _run_device(x, force, trace=False)
    diffusion = np.broadcast_to(_DSIG, x.shape)
    return drift, diffusion
